# revision 1
# baseline (speedup 1.0000x reference)
"""TopK autoencoder (B=4096, D=1024, F=32768, K=64) on 8 Trainium2 NeuronCores.

Strategy: data-parallel over batch (512 rows/core). Per core, per 128-row tile:
  A) fp32 encoder matmul (PE), relu fused on ACT, spilled to DRAM; fused
     per-group (32 elems) running max on DVE.
  B) exact top-K: group maxima packed as (bf16-value-bits << 16 | group-id)
     so max8/match_replace rounds are tie-free; top-80 groups gathered from
     the spilled pre-activations by indirect DMA; candidates packed the same
     way with element tags; top-80 (value, index) pairs extracted; exact fp32
     values re-fetched by point-gather; K-th largest = exact threshold.
  C) decode: gather the selected W_dec rows (fp16) by index and accumulate
     w_k * row_k on the PE via diagonal-matrix matmuls into PSUM.
b_dec is handled exactly on the host (x - b_dec, + b_dec at the end); a
nonzero b_enc is folded in as an extra contraction tile (zero here).
"""
import sys
sys.path.insert(0, '/opt/trn_rl_repo')
import numpy as np
import concourse.bass as bass
import concourse.mybir as mybir
from concourse import bacc
from concourse.bass import ts, ds
from concourse.tile import TileContext
from concourse.masks import make_identity
from concourse.bass_utils import run_bass_kernel_spmd

f32 = mybir.dt.float32
f16 = mybir.dt.float16
bf16 = mybir.dt.bfloat16
u16 = mybir.dt.uint16
u32 = mybir.dt.uint32
i32 = mybir.dt.int32
Alu = mybir.AluOpType

B, D, F, K = 4096, 1024, 32768, 64
N_CORES = 8
GE = 32     # group size
NP = 80     # candidate groups / extracted pairs per row
PB = 8      # decode gather block

_CACHE = {}


def build(DX, DO, FF, BL, n_cores=N_CORES, reps=1):
    """Per-core kernel. DX: contraction dim (may include bias tile), DO: output dim."""
    KT = DX // 128
    NT = BL // 128
    FC = 512
    NFC = FF // FC
    NG = FF // GE
    GPC = FC // GE

    nc = bacc.Bacc("TRN2", target_bir_lowering=False, debug=False, num_devices=n_cores)
    xt = nc.dram_tensor("xt", [3, DX, BL], f16, kind="ExternalInput")
    wencT = nc.dram_tensor("wencT", [2, DX, FF], f16, kind="ExternalInput")
    wdecT = nc.dram_tensor("wdecT", [FF, DO], f16, kind="ExternalInput")
    out = nc.dram_tensor("out", [BL, DO], f32, kind="ExternalOutput")

    wencT_r = wencT.ap().rearrange("s (k p) f -> p s k f", p=128)
    xt_r = xt.ap().rearrange("s (k p) b -> p s k b", p=128)

    with TileContext(nc) as tc:
        with (
            tc.tile_pool(name="dram", bufs=1, space="DRAM") as dpool,
            tc.tile_pool(name="xt_sb", bufs=1) as xpool,
            tc.tile_pool(name="const", bufs=1) as kpool,
            tc.tile_pool(name="wenc", bufs=2) as wpool,
            tc.tile_pool(name="apsum", bufs=3, space="PSUM") as apsum,
            tc.tile_pool(name="abounce", bufs=4) as apool,
            tc.tile_pool(name="gbuf", bufs=2) as gpool,
            tc.tile_pool(name="cand", bufs=1) as cpool,
            tc.tile_pool(name="pack", bufs=1) as ppool,
            tc.tile_pool(name="small", bufs=4) as spool,
            tc.tile_pool(name="wdecg", bufs=2) as wgpool,
            tc.tile_pool(name="dpsum", bufs=2, space="PSUM") as dpsum,
            tc.tile_pool(name="cout", bufs=1) as opool,
        ):
            preD = dpool.tile([BL, FF], f32)
            preD_g = preD[:, :].rearrange("b (g e) -> (b g) e", e=GE)
            preD_f = preD[:, :].rearrange("b (f o) -> (b f) o", o=1)

            xt_sb = xpool.tile([128, 3, KT, BL], f16)
            nc.sync.dma_start(out=xt_sb[:], in_=xt_r)
            ident = kpool.tile([128, 128], f16)
            make_identity(nc, ident[:])
            gid = kpool.tile([128, NG], i32)
            nc.gpsimd.iota(gid[:], pattern=[[1, NG]], base=0, channel_multiplier=0)
            tagi = kpool.tile([128, NP, GE], i32)
            nc.gpsimd.iota(tagi[:], pattern=[[0, NP], [1, GE]], base=0,
                           channel_multiplier=0)

            def pair_phase_a(t0):
                # two row-tiles share each W chunk load (halves W traffic)
                Gs = [gpool.tile([128, NG], f32, name=f"G{_i}") for _i in range(2)]
                for fc in range(NFC):
                    w = wpool.tile([128, 2, KT, FC], f16, name="w")
                    nc.sync.dma_start(out=w[:], in_=wencT_r[:, :, :, ds(fc * FC, FC)])
                    for ti in range(2):
                        t = t0 + ti
                        rows = ts(t, 128)
                        ps = apsum.tile([128, FC], f32, name="ps")
                        # (xh, Wh), (xh*2^-11, Wl*2^11), (xl, Wh)
                        for gi, (xs_, ws_) in enumerate(((0, 0), (1, 1), (2, 0))):
                            for k in range(KT):
                                nc.tensor.matmul(
                                    ps[:], lhsT=xt_sb[:, xs_, k, rows], rhs=w[:, ws_, k, :],
                                    start=(gi == 0 and k == 0), stop=(gi == 2 and k == KT - 1),
                                )
                        a = apool.tile([128, FC], f32, name="a")
                        nc.scalar.activation(a[:], ps[:], mybir.ActivationFunctionType.Relu)
                        nc.sync.dma_start(out=preD[rows, ds(fc * FC, FC)], in_=a[:])
                        av = a[:, :].rearrange("p (g e) -> p g e", e=GE)
                        nc.vector.reduce_max(
                            out=Gs[ti][:, ds(fc * GPC, GPC)], in_=av,
                            axis=mybir.AxisListType.X)
                return Gs

            def tile_body(t, G):
                rows = ts(t, 128)
                # ---------- Phase B ----------
                gbf = gpool.tile([128, NG], bf16, name="gbf")
                nc.vector.tensor_copy(gbf[:], G[:])
                gpk = ppool.tile([128, NG], u32, name="gpk")
                nc.vector.tensor_copy(gpk[:], gbf[:, :].bitcast(u16))
                nc.vector.tensor_scalar(out=gpk[:], in0=gpk[:], scalar1=16,
                                        scalar2=None, op0=Alu.logical_shift_left)
                nc.vector.tensor_tensor(out=gpk[:], in0=gpk[:], in1=gid[:, :].bitcast(u32),
                                        op=Alu.bitwise_or)
                gpkf = gpk[:, :].bitcast(f32)
                gtop = spool.tile([128, NP], f32, name="gtop")
                for r in range(NP // 8):
                    mv = gtop[:, ds(r * 8, 8)]
                    nc.vector.max(out=mv, in_=gpkf)
                    if r < NP // 8 - 1:
                        nc.vector.match_replace(out=gpkf, in_to_replace=mv,
                                                in_values=gpkf, imm_value=0.0)
                gsel = spool.tile([128, NP], u32, name="gsel")
                nc.vector.tensor_scalar(out=gsel[:], in0=gtop[:, :].bitcast(u32),
                                        scalar1=0xFFFF, scalar2=None, op0=Alu.bitwise_and)
                goff = spool.tile([128, NP], i32, name="goff")
                nc.gpsimd.iota(goff[:], pattern=[[0, NP]], base=t * 128 * NG,
                               channel_multiplier=NG)
                nc.vector.tensor_tensor(out=goff[:], in0=goff[:], in1=gsel[:, :].bitcast(i32),
                                        op=Alu.add)
                cand = cpool.tile([128, NP, GE], f32, name="cand")
                for k in range(NP):
                    nc.gpsimd.indirect_dma_start(
                        out=cand[:, k, :], out_offset=None, in_=preD_g,
                        in_offset=bass.IndirectOffsetOnAxis(ap=goff[:, k:k + 1], axis=0),
                    )
                gsel_b = gsel[:, :].rearrange("p (n o) -> p n o", o=1).to_broadcast([128, NP, GE])
                tagm = ppool.tile([128, NP, GE], u32, name="tagm")
                nc.vector.tensor_scalar(out=tagm[:], in0=gsel_b, scalar1=GE,
                                        scalar2=None, op0=Alu.mult)
                nc.vector.tensor_tensor(out=tagm[:], in0=tagm[:], in1=tagi[:, :, :].bitcast(u32),
                                        op=Alu.add)
                cbf = cpool.tile([128, NP * GE], bf16, name="cbf")
                cand2 = cand[:, :, :].rearrange("p n e -> p (n e)")
                nc.vector.tensor_copy(cbf[:], cand2)
                cpk = cpool.tile([128, NP * GE], u32, name="cpk")
                nc.vector.tensor_copy(cpk[:], cbf[:, :].bitcast(u16))
                nc.vector.tensor_scalar(out=cpk[:], in0=cpk[:], scalar1=16,
                                        scalar2=None, op0=Alu.logical_shift_left)
                tagm2 = tagm[:, :, :].rearrange("p n e -> p (n e)")
                nc.vector.tensor_tensor(out=cpk[:], in0=cpk[:], in1=tagm2, op=Alu.bitwise_or)
                # exact t* from candidate values (destructive rounds on a copy)
                vr = cpool.tile([128, NP * GE], f32, name="vr")
                nc.vector.tensor_copy(vr[:], cand2)
                mvf = None
                for r in range(K // 8):
                    mvf = spool.tile([128, 8], f32, name="mvf")
                    nc.vector.max(out=mvf[:], in_=vr[:])
                    if r < K // 8 - 1:
                        nc.vector.match_replace(out=vr[:], in_to_replace=mvf[:],
                                                in_values=vr[:], imm_value=0.0)
                tstar = spool.tile([128, 1], f32, name="tstar")
                nc.vector.tensor_copy(tstar[:], mvf[:, 7:8])
                # mask packed array to the exact selection, then extract pairs
                cpkf = cpk[:, :].bitcast(f32)
                nc.vector.scalar_tensor_tensor(
                    out=cpkf, in0=cand2, scalar=tstar[:], in1=cpkf,
                    op0=Alu.is_ge, op1=Alu.mult,
                )
                pk = spool.tile([128, K], f32, name="pk")
                for r in range(K // 8):
                    mv = pk[:, ds(r * 8, 8)]
                    nc.vector.max(out=mv, in_=cpkf)
                    if r < K // 8 - 1:
                        nc.vector.match_replace(out=cpkf, in_to_replace=mv,
                                                in_values=cpkf, imm_value=0.0)
                fsel = spool.tile([128, K], u32, name="fsel")
                nc.vector.tensor_scalar(out=fsel[:], in0=pk[:, :].bitcast(u32),
                                        scalar1=0xFFFF, scalar2=None, op0=Alu.bitwise_and)
                wbits = spool.tile([128, K], u32, name="wbits")
                nc.vector.tensor_scalar(out=wbits[:], in0=pk[:, :].bitcast(u32),
                                        scalar1=16, scalar2=None,
                                        op0=Alu.logical_shift_right)
                wnarrow = spool.tile([128, K], u16, name="wnarrow")
                nc.vector.tensor_copy(wnarrow[:], wbits[:])
                wsel = spool.tile([128, K], f32, name="wsel")
                nc.vector.tensor_copy(wsel[:], wnarrow[:, :].bitcast(bf16))

                # ---------- Phase C ----------
                ND2 = max(1, DO // 512)
                DW = DO // ND2
                psD = [dpsum.tile([128, DW], f32, name=f"psD{_h}") for _h in range(ND2)]
                for blk in range(K // PB):
                    wg = wgpool.tile([128, PB, DO], f16, name="wg")
                    for j in range(PB):
                        k = blk * PB + j
                        nc.gpsimd.indirect_dma_start(
                            out=wg[:, j, :], out_offset=None, in_=wdecT[:, :],
                            in_offset=bass.IndirectOffsetOnAxis(ap=fsel[:, k:k + 1], axis=0),
                        )
                    for j in range(PB):
                        k = blk * PB + j
                        dg = apool.tile([128, 128], f16, name="dg")
                        nc.vector.tensor_scalar(out=dg[:], in0=ident[:],
                                                scalar1=wsel[:, k:k + 1], scalar2=None,
                                                op0=Alu.mult)
                        first = (k == 0)
                        last = (k == K - 1)
                        for h in range(ND2):
                            nc.tensor.matmul(psD[h][:], lhsT=dg[:],
                                             rhs=wg[:, j, ds(h * DW, DW)],
                                             start=first, stop=last)
                co = opool.tile([128, DO], f32, name="co")
                for h in range(ND2):
                    nc.vector.tensor_copy(co[:, ds(h * DW, DW)], psD[h][:])
                nc.sync.dma_start(out=out.ap()[rows, :], in_=co[:])

            def full_pass():
                for t0 in range(0, NT, 2):
                    Gs = pair_phase_a(t0)
                    tile_body(t0, Gs[0])
                    tile_body(t0 + 1, Gs[1])

            if reps > 1:
                with tc.For_i(0, reps, 1):
                    full_pass()
            else:
                full_pass()

    nc.compile()
    return nc


def get_kernel(DX, reps=1):
    key = (DX, reps)
    if key not in _CACHE:
        _CACHE[key] = build(DX, D, F, B // N_CORES, N_CORES, reps=reps)
    return _CACHE[key]


def prep_in_maps(x, W_enc, b_enc, W_dec, b_dec):
    BL = B // N_CORES
    xs = (x - b_dec).astype(np.float32)
    wencT = np.ascontiguousarray(W_enc.T.astype(np.float32))   # [D, F]
    if np.any(b_enc):
        # fold b_enc in as one extra 128-row contraction tile
        DX = D + 128
        xa = np.zeros((B, DX), np.float32)
        xa[:, :D] = xs
        xa[:, D] = 1.0
        wa = np.zeros((DX, F), np.float32)
        wa[:D] = wencT
        wa[D] = b_enc
        xs, wencT = xa, wa
    else:
        DX = D
    xst = np.ascontiguousarray(xs.T)                            # [DX, B]
    wdecT = np.ascontiguousarray(W_dec.T).astype(np.float16)    # [F, D]
    # fp16 3-split operands (exact to ~2^-22): hi, scaled-lo, residual
    wh = wencT.astype(np.float16)
    wls = ((wencT - wh.astype(np.float32)) * 2.0 ** 11).astype(np.float16)
    wenc2 = np.stack([wh, wls])                                 # [2, DX, F]
    xh = xst.astype(np.float16)
    xhs = (xh.astype(np.float32) * 2.0 ** -11).astype(np.float16)
    xl = (xst - xh.astype(np.float32)).astype(np.float16)
    xt3 = np.stack([xh, xhs, xl])                               # [3, DX, B]
    in_maps = [{
        "xt": np.ascontiguousarray(xt3[:, :, c * BL:(c + 1) * BL]),
        "wencT": wenc2,
        "wdecT": wdecT,
    } for c in range(N_CORES)]
    return in_maps, DX


def kernel(x, W_enc, b_enc, W_dec, b_dec):
    x = np.asarray(x, np.float32)
    W_enc = np.asarray(W_enc, np.float32)
    b_enc = np.asarray(b_enc, np.float32)
    W_dec = np.asarray(W_dec, np.float32)
    b_dec = np.asarray(b_dec, np.float32)
    in_maps, DX = prep_in_maps(x, W_enc, b_enc, W_dec, b_dec)
    nc = get_kernel(DX)
    res = run_bass_kernel_spmd(nc, in_maps, list(range(N_CORES)))
    y = np.concatenate([res.results[c]["out"] for c in range(N_CORES)], axis=0)
    return (y + b_dec).astype(np.float32)



# revision 7
# speedup vs baseline: 5.6184x; 5.6184x over previous
"""TopK autoencoder (B=4096, D=1024, F=32768, K=64) on 8 Trainium2 NeuronCores.

Strategy: data-parallel over batch (512 rows/core). Per core, per 128-row tile:
  A) encoder matmul in mixed precision: hi pass (fp16 x, fp16 W) into PSUM1;
     two correction passes in fp8 DoubleRow (2 contraction slices per MM) into
     PSUM2 at a common 2^11 operand scale; DVE combines ps1 + 2^-11*ps2, ACT
     applies relu, result spilled to DRAM; fused per-group (32 elems) running
     max on DVE. End-to-end rel err of this scheme vs exact: ~5e-3.
  B) exact top-K on the combined values: group maxima packed as
     (bf16-value-bits << 16 | group-id) so max8/match_replace rounds are
     tie-free; top-80 groups gathered from the spilled pre-activations by
     indirect DMA; candidates packed the same way with element tags; K-th
     largest of candidate values = exact threshold; masked extraction gives
     the exact top-64 (value, index) pairs.
  C) decode: gather the selected W_dec rows (fp16) by index and accumulate
     w_k * row_k on the PE via diagonal-matrix matmuls into PSUM.
b_dec is handled exactly on the host (x - b_dec, + b_dec at the end); a
nonzero b_enc is folded in as an extra contraction tile (zero here).
"""
import sys
sys.path.insert(0, '/opt/trn_rl_repo')
import numpy as np
import ml_dtypes
import concourse.bass as bass
import concourse.mybir as mybir
from concourse import bacc
from concourse.bass import ts, ds
from concourse.tile import TileContext
from concourse.masks import make_identity
from concourse.bass_utils import run_bass_kernel_spmd

f32 = mybir.dt.float32
f16 = mybir.dt.float16
bf16 = mybir.dt.bfloat16
f8 = mybir.dt.float8e4
u16 = mybir.dt.uint16
u32 = mybir.dt.uint32
i32 = mybir.dt.int32
Alu = mybir.AluOpType
DR = mybir.MatmulPerfMode.DoubleRow
E4NP = ml_dtypes.float8_e4m3

B, D, F, K = 4096, 1024, 32768, 64
N_CORES = 8
GE = 32     # group size
NP = 80     # candidate groups / extracted pairs per row
PB = 8      # decode gather block
CSCALE = 2.0 ** -11   # correction-pass PSUM scale

_CACHE = {}


def build(DX, DO, FF, BL, n_cores=N_CORES, reps=1):
    """Per-core kernel. DX: contraction dim (may include bias tiles), DO: output dim."""
    KT = DX // 128
    assert KT % 2 == 0
    FC = 512
    NFC = FF // FC
    NT = BL // 128
    NG = FF // GE
    GPC = FC // GE

    nc = bacc.Bacc("TRN2", target_bir_lowering=False, debug=False, num_devices=n_cores)
    xt = nc.dram_tensor("xt", [DX, BL], f16, kind="ExternalInput")
    x8 = nc.dram_tensor("x8", [2, DX, BL], f8, kind="ExternalInput")
    wh = nc.dram_tensor("wh", [DX, FF], f16, kind="ExternalInput")
    wl8 = nc.dram_tensor("wl8", [DX, FF], f8, kind="ExternalInput")
    wdecT = nc.dram_tensor("wdecT", [FF, DO], f16, kind="ExternalInput")
    out = nc.dram_tensor("out", [BL, DO], f32, kind="ExternalOutput")

    wh_r = wh.ap().rearrange("(k p) f -> p k f", p=128)
    wl8_r = wl8.ap().rearrange("(k p) f -> p k f", p=128)
    xt_r = xt.ap().rearrange("(k p) b -> p k b", p=128)
    x8_r = x8.ap().rearrange("s (k p) b -> p s k b", p=128)

    with TileContext(nc) as tc:
        with (
            tc.tile_pool(name="dram", bufs=1, space="DRAM") as dpool,
            tc.tile_pool(name="xt_sb", bufs=1) as xpool,
            tc.tile_pool(name="const", bufs=1) as kpool,
            tc.tile_pool(name="wenc", bufs=2) as wpool,
            tc.tile_pool(name="wenc8", bufs=2) as w8pool,
            tc.tile_pool(name="apsum", bufs=2, space="PSUM") as apsum,
            tc.tile_pool(name="cpsum", bufs=2, space="PSUM") as cpsum,
            tc.tile_pool(name="abounce", bufs=4) as apool,
            tc.tile_pool(name="gbuf", bufs=2) as gpool,
            tc.tile_pool(name="cand", bufs=1) as cpool,
            tc.tile_pool(name="pack", bufs=1) as ppool,
            tc.tile_pool(name="small", bufs=4) as spool,
            tc.tile_pool(name="wdecg", bufs=2) as wgpool,
            tc.tile_pool(name="dpsum", bufs=2, space="PSUM") as dpsum,
            tc.tile_pool(name="cout", bufs=1) as opool,
        ):
            preD = dpool.tile([BL, FF], f32)
            preD_g = preD[:, :].rearrange("b (g e) -> (b g) e", e=GE)

            xh_sb = xpool.tile([128, KT, BL], f16)
            nc.sync.dma_start(out=xh_sb[:], in_=xt_r)
            x8_sb = xpool.tile([128, 2, KT, BL], f8)
            nc.sync.dma_start(out=x8_sb[:], in_=x8_r)
            ident = kpool.tile([128, 128], f16)
            make_identity(nc, ident[:])
            gid = kpool.tile([128, NG], i32)
            nc.gpsimd.iota(gid[:], pattern=[[1, NG]], base=0, channel_multiplier=0)
            tagi = kpool.tile([128, NP, GE], i32)
            nc.gpsimd.iota(tagi[:], pattern=[[0, NP], [1, GE]], base=0,
                           channel_multiplier=0)

            def pair_phase_a(t0):
                # two row-tiles share each W chunk load (halves W traffic)
                Gs = [gpool.tile([128, NG], f32, name=f"G{_i}") for _i in range(2)]
                for fc in range(NFC):
                    w = wpool.tile([128, KT, FC], f16, name="w")
                    nc.sync.dma_start(out=w[:], in_=wh_r[:, :, ds(fc * FC, FC)])
                    l8 = w8pool.tile([128, KT, FC], f8, name="l8")
                    nc.sync.dma_start(out=l8[:], in_=wl8_r[:, :, ds(fc * FC, FC)])
                    h8 = w8pool.tile([128, KT, FC], f8, name="h8")
                    nc.vector.tensor_copy(h8[:], w[:])
                    for ti in range(2):
                        t = t0 + ti
                        rows = ts(t, 128)
                        ps1 = apsum.tile([128, FC], f32, name="ps1")
                        for k in range(KT):
                            nc.tensor.matmul(
                                ps1[:], lhsT=xh_sb[:, k, rows], rhs=w[:, k, :],
                                start=(k == 0), stop=(k == KT - 1),
                            )
                        ps2 = cpsum.tile([128, FC], f32, name="ps2")
                        for kp in range(KT // 2):
                            nc.tensor.matmul(
                                ps2[:], lhsT=x8_sb[:, 0, ds(2 * kp, 2), rows],
                                rhs=h8[:, ds(2 * kp, 2), :],
                                start=(kp == 0), stop=False, perf_mode=DR,
                            )
                        for kp in range(KT // 2):
                            nc.tensor.matmul(
                                ps2[:], lhsT=x8_sb[:, 1, ds(2 * kp, 2), rows],
                                rhs=l8[:, ds(2 * kp, 2), :],
                                start=False, stop=(kp == KT // 2 - 1), perf_mode=DR,
                            )
                        # only one PSUM input per DVE op: ACT scaled-copies ps2,
                        # DVE adds ps1. relu is skipped on purpose — top-K values
                        # are all positive, so selection and decode are unchanged
                        # on unrectified pre-activations.
                        c2 = apool.tile([128, FC], f32, name="c2")
                        nc.scalar.activation(c2[:], ps2[:],
                                             mybir.ActivationFunctionType.Copy,
                                             scale=CSCALE)
                        a = apool.tile([128, FC], f32, name="a")
                        nc.vector.tensor_tensor(out=a[:], in0=c2[:], in1=ps1[:],
                                                op=Alu.add)
                        nc.sync.dma_start(out=preD[rows, ds(fc * FC, FC)], in_=a[:])
                        av = a[:, :].rearrange("p (g e) -> p g e", e=GE)
                        nc.vector.reduce_max(
                            out=Gs[ti][:, ds(fc * GPC, GPC)], in_=av,
                            axis=mybir.AxisListType.X)
                return Gs

            def tile_body(t, G):
                rows = ts(t, 128)
                # ---------- Phase B ----------
                gbf = gpool.tile([128, NG], bf16, name="gbf")
                nc.vector.tensor_copy(gbf[:], G[:])
                gpk = ppool.tile([128, NG], u32, name="gpk")
                nc.vector.tensor_copy(gpk[:], gbf[:, :].bitcast(u16))
                nc.vector.tensor_scalar(out=gpk[:], in0=gpk[:], scalar1=16,
                                        scalar2=None, op0=Alu.logical_shift_left)
                nc.vector.tensor_tensor(out=gpk[:], in0=gpk[:], in1=gid[:, :].bitcast(u32),
                                        op=Alu.bitwise_or)
                gpkf = gpk[:, :].bitcast(f32)
                gtop = spool.tile([128, NP], f32, name="gtop")
                for r in range(NP // 8):
                    mv = gtop[:, ds(r * 8, 8)]
                    nc.vector.max(out=mv, in_=gpkf)
                    if r < NP // 8 - 1:
                        nc.vector.match_replace(out=gpkf, in_to_replace=mv,
                                                in_values=gpkf, imm_value=0.0)
                gsel = spool.tile([128, NP], u32, name="gsel")
                nc.vector.tensor_scalar(out=gsel[:], in0=gtop[:, :].bitcast(u32),
                                        scalar1=0xFFFF, scalar2=None, op0=Alu.bitwise_and)
                goff = spool.tile([128, NP], i32, name="goff")
                nc.gpsimd.iota(goff[:], pattern=[[0, NP]], base=t * 128 * NG,
                               channel_multiplier=NG)
                nc.vector.tensor_tensor(out=goff[:], in0=goff[:], in1=gsel[:, :].bitcast(i32),
                                        op=Alu.add)
                cand = cpool.tile([128, NP, GE], f32, name="cand")
                for k in range(NP):
                    nc.gpsimd.indirect_dma_start(
                        out=cand[:, k, :], out_offset=None, in_=preD_g,
                        in_offset=bass.IndirectOffsetOnAxis(ap=goff[:, k:k + 1], axis=0),
                    )
                gsel_b = gsel[:, :].rearrange("p (n o) -> p n o", o=1).to_broadcast([128, NP, GE])
                tagm = ppool.tile([128, NP, GE], u32, name="tagm")
                nc.vector.tensor_scalar(out=tagm[:], in0=gsel_b, scalar1=GE,
                                        scalar2=None, op0=Alu.mult)
                nc.vector.tensor_tensor(out=tagm[:], in0=tagm[:], in1=tagi[:, :, :].bitcast(u32),
                                        op=Alu.add)
                cbf = cpool.tile([128, NP * GE], bf16, name="cbf")
                cand2 = cand[:, :, :].rearrange("p n e -> p (n e)")
                nc.vector.tensor_copy(cbf[:], cand2)
                cpk = cpool.tile([128, NP * GE], u32, name="cpk")
                nc.vector.tensor_copy(cpk[:], cbf[:, :].bitcast(u16))
                nc.vector.tensor_scalar(out=cpk[:], in0=cpk[:], scalar1=16,
                                        scalar2=None, op0=Alu.logical_shift_left)
                tagm2 = tagm[:, :, :].rearrange("p n e -> p (n e)")
                nc.vector.tensor_tensor(out=cpk[:], in0=cpk[:], in1=tagm2, op=Alu.bitwise_or)
                # exact t* from candidate values (destructive rounds on a copy)
                vr = cpool.tile([128, NP * GE], f32, name="vr")
                nc.vector.tensor_copy(vr[:], cand2)
                mvf = None
                for r in range(K // 8):
                    mvf = spool.tile([128, 8], f32, name="mvf")
                    nc.vector.max(out=mvf[:], in_=vr[:])
                    if r < K // 8 - 1:
                        nc.vector.match_replace(out=vr[:], in_to_replace=mvf[:],
                                                in_values=vr[:], imm_value=0.0)
                tstar = spool.tile([128, 1], f32, name="tstar")
                nc.vector.tensor_copy(tstar[:], mvf[:, 7:8])
                # mask packed array to the exact selection, then extract pairs
                cpkf = cpk[:, :].bitcast(f32)
                nc.vector.scalar_tensor_tensor(
                    out=cpkf, in0=cand2, scalar=tstar[:], in1=cpkf,
                    op0=Alu.is_ge, op1=Alu.mult,
                )
                pk = spool.tile([128, K], f32, name="pk")
                for r in range(K // 8):
                    mv = pk[:, ds(r * 8, 8)]
                    nc.vector.max(out=mv, in_=cpkf)
                    if r < K // 8 - 1:
                        nc.vector.match_replace(out=cpkf, in_to_replace=mv,
                                                in_values=cpkf, imm_value=0.0)
                fsel = spool.tile([128, K], u32, name="fsel")
                nc.vector.tensor_scalar(out=fsel[:], in0=pk[:, :].bitcast(u32),
                                        scalar1=0xFFFF, scalar2=None, op0=Alu.bitwise_and)
                wbits = spool.tile([128, K], u32, name="wbits")
                nc.vector.tensor_scalar(out=wbits[:], in0=pk[:, :].bitcast(u32),
                                        scalar1=16, scalar2=None,
                                        op0=Alu.logical_shift_right)
                wnarrow = spool.tile([128, K], u16, name="wnarrow")
                nc.vector.tensor_copy(wnarrow[:], wbits[:])
                wsel = spool.tile([128, K], f32, name="wsel")
                nc.vector.tensor_copy(wsel[:], wnarrow[:, :].bitcast(bf16))

                # ---------- Phase C ----------
                ND2 = max(1, DO // 512)
                DW = DO // ND2
                psD = [dpsum.tile([128, DW], f32, name=f"psD{_h}") for _h in range(ND2)]
                for blk in range(K // PB):
                    wg = wgpool.tile([128, PB, DO], f16, name="wg")
                    for j in range(PB):
                        k = blk * PB + j
                        nc.gpsimd.indirect_dma_start(
                            out=wg[:, j, :], out_offset=None, in_=wdecT[:, :],
                            in_offset=bass.IndirectOffsetOnAxis(ap=fsel[:, k:k + 1], axis=0),
                        )
                    for j in range(PB):
                        k = blk * PB + j
                        dg = apool.tile([128, 128], f16, name="dg")
                        nc.vector.tensor_scalar(out=dg[:], in0=ident[:],
                                                scalar1=wsel[:, k:k + 1], scalar2=None,
                                                op0=Alu.mult)
                        first = (k == 0)
                        last = (k == K - 1)
                        for h in range(ND2):
                            nc.tensor.matmul(psD[h][:], lhsT=dg[:],
                                             rhs=wg[:, j, ds(h * DW, DW)],
                                             start=first, stop=last)
                co = opool.tile([128, DO], f32, name="co")
                for h in range(ND2):
                    nc.vector.tensor_copy(co[:, ds(h * DW, DW)], psD[h][:])
                nc.sync.dma_start(out=out.ap()[rows, :], in_=co[:])

            def full_pass():
                for t0 in range(0, NT, 2):
                    Gs = pair_phase_a(t0)
                    tile_body(t0, Gs[0])
                    tile_body(t0 + 1, Gs[1])

            if reps > 1:
                with tc.For_i(0, reps, 1):
                    full_pass()
            else:
                full_pass()

    nc.compile()
    return nc


def get_kernel(DX, reps=1):
    key = (DX, reps)
    if key not in _CACHE:
        _CACHE[key] = build(DX, D, F, B // N_CORES, N_CORES, reps=reps)
    return _CACHE[key]


def prep_in_maps(x, W_enc, b_enc, W_dec, b_dec):
    BL = B // N_CORES
    xs = (x - b_dec).astype(np.float32)
    wencT = np.ascontiguousarray(W_enc.T.astype(np.float32))   # [D, F]
    if np.any(b_enc):
        # fold b_enc in as extra contraction tiles (keep KT even for DoubleRow)
        DX = D + 256
        xa = np.zeros((B, DX), np.float32)
        xa[:, :D] = xs
        xa[:, D] = 1.0
        wa = np.zeros((DX, F), np.float32)
        wa[:D] = wencT
        wa[D] = b_enc
        xs, wencT = xa, wa
    else:
        DX = D
    xst = np.ascontiguousarray(xs.T)                            # [DX, B]
    wdecT = np.ascontiguousarray(W_dec.T).astype(np.float16)    # [F, D]
    # mixed-precision split: fp16 hi; fp8 residual/lo at common 2^11 scale
    wh = wencT.astype(np.float16)
    wl8 = ((wencT - wh.astype(np.float32)) * 2.0 ** 11).astype(E4NP)
    xh = xst.astype(np.float16)
    xl8 = ((xst - xh.astype(np.float32)) * 2.0 ** 11).astype(E4NP)
    xh8 = xh.astype(E4NP)
    x8 = np.stack([xl8, xh8])                                   # [2, DX, B]
    in_maps = [{
        "xt": np.ascontiguousarray(xh[:, c * BL:(c + 1) * BL]),
        "x8": np.ascontiguousarray(x8[:, :, c * BL:(c + 1) * BL]),
        "wh": wh,
        "wl8": wl8,
        "wdecT": wdecT,
    } for c in range(N_CORES)]
    return in_maps, DX


def kernel(x, W_enc, b_enc, W_dec, b_dec):
    x = np.asarray(x, np.float32)
    W_enc = np.asarray(W_enc, np.float32)
    b_enc = np.asarray(b_enc, np.float32)
    W_dec = np.asarray(W_dec, np.float32)
    b_dec = np.asarray(b_dec, np.float32)
    in_maps, DX = prep_in_maps(x, W_enc, b_enc, W_dec, b_dec)
    nc = get_kernel(DX)
    res = run_bass_kernel_spmd(nc, in_maps, list(range(N_CORES)))
    y = np.concatenate([res.results[c]["out"] for c in range(N_CORES)], axis=0)
    return (y + b_dec).astype(np.float32)
